# revision 3
# baseline (speedup 1.0000x reference)
"""Trainium2 Bass kernel for nn_DecoderLSTM_8641474200440.

Computation (see original nn.Module):
  - Only the last 128 elements of each row's flattened (F,S) block feed the
    decoder ("x_last", shape (B*N, 128) after the torch reshape chain).
  - gx = x_last @ W_ih.T + b_ih + b_hh is constant across the 24 decode steps.
  - 24-step LSTM recurrence over 32768 independent rows, hidden 256.
  - y_t = h_t @ conv_w + conv_b  -> output (B, N, 24), returned twice.

Device strategy (8 cores, data-parallel over the 32768 rows, 4096 rows/core):
  - "transposed" layout: feature dims on SBUF partitions, rows on the free dim.
  - state tile sT = [h (2x128) | x_last (1x128)] in bf16; combined weight
    Wc = [W_hh | W_ih] so each step's gate preactivation is ONE K=384 matmul
    sweep (the x contribution rides in the matmul; no separate gx buffer).
  - gates: PSUM fp32 -> ScalarE sigmoid/tanh (bias folded into ACT's affine),
    cell update on VectorE, c kept fp32, h stored bf16.
  - y_t via M=1 matmuls against conv_w, +conv_b via ACT Identity bias.
"""

import os
import sys
import numpy as np

if "/opt/trn_rl_repo" not in sys.path:
    sys.path.insert(0, "/opt/trn_rl_repo")

B, F, N, S = 32, 128, 1024, 12
H, STEPS = 256, 24
BN = B * N
NCORES = 8
R = BN // NCORES          # rows per core = 4096
CH = 1024                 # free-dim chunk for ACT/DVE ops
NCH = R // CH
N512 = R // 512

_CACHE = {}


def _build_nc():
    import concourse.bass as bass
    import concourse.mybir as mybir
    import concourse.tile as tile
    from concourse import bacc

    f32 = mybir.dt.float32
    bf16 = mybir.dt.bfloat16
    AF = mybir.ActivationFunctionType
    OP = mybir.AluOpType

    nc = bacc.Bacc(None, target_bir_lowering=False)

    xT = nc.dram_tensor("xT", [F, R], f32, kind="ExternalInput")
    wT = nc.dram_tensor("wT", [384, 4 * H], f32, kind="ExternalInput")
    biasd = nc.dram_tensor("biasd", [4 * H], f32, kind="ExternalInput")
    convw = nc.dram_tensor("convw", [H], f32, kind="ExternalInput")
    convb = nc.dram_tensor("convb", [1, 1], f32, kind="ExternalInput")
    y = nc.dram_tensor("y", [STEPS, R], f32, kind="ExternalOutput")

    with tile.TileContext(nc) as tc:
        with (
            tc.tile_pool(name="const", bufs=1) as const,
            tc.tile_pool(name="stage", bufs=1) as stage,
            tc.tile_pool(name="gps", bufs=3, space="PSUM") as gps,
            tc.tile_pool(name="cps", bufs=2, space="PSUM") as cpsp,
            tc.tile_pool(name="gact", bufs=2) as gact,
            tc.tile_pool(name="dvet", bufs=2) as dvet,
            tc.tile_pool(name="ysbp", bufs=4) as ysbp,
        ):
            # ---- load + cast constants ----
            wstage = stage.tile([128, 3, 4 * H], f32)
            nc.sync.dma_start(wstage[:], wT.rearrange("(kc p) m -> p kc m", p=128))
            wc = const.tile([128, 3, 4 * H], bf16)
            nc.vector.tensor_copy(wc[:], wstage[:])

            xstage = stage.tile([128, R], f32)
            nc.sync.dma_start(xstage[:], xT[:])
            sT = const.tile([128, 3, R], bf16)      # [h0 | h1 | x_last]
            nc.vector.tensor_copy(sT[:, 2], xstage[:])
            nc.gpsimd.memset(sT[:, 0:2], 0.0)

            cT = const.tile([128, 2, R], f32)
            nc.gpsimd.memset(cT[:], 0.0)

            biasT = const.tile([128, 8], f32)
            nc.sync.dma_start(biasT[:], biasd.rearrange("(o p) -> p o", p=128))

            cwstage = stage.tile([128, 2], f32)
            nc.sync.dma_start(cwstage[:], convw.rearrange("(o p) -> p o", p=128))
            cwT = const.tile([128, 2], bf16)
            nc.vector.tensor_copy(cwT[:], cwstage[:])

            cbT = const.tile([1, 1], f32)
            nc.sync.dma_start(cbT[:], convb[:])

            # ---- recurrence ----
            for t in range(STEPS):
                for ch in range(NCH):
                    c0 = ch * CH
                    for hb in range(2):
                        acts = []
                        for gi, m in enumerate((hb, 2 + hb, 4 + hb, 6 + hb)):
                            ps = gps.tile([128, CH], f32, tag="gps")
                            for k in range(3):
                                for h2 in range(CH // 512):
                                    nc.tensor.matmul(
                                        ps[:, h2 * 512:(h2 + 1) * 512],
                                        wc[:, k, m * 128:(m + 1) * 128],
                                        sT[:, k, c0 + h2 * 512: c0 + (h2 + 1) * 512],
                                        start=(k == 0),
                                        stop=(k == 2),
                                    )
                            func = AF.Tanh if gi == 2 else AF.Sigmoid
                            dt_g = f32 if gi == 1 else bf16
                            at = gact.tile([128, CH], dt_g, tag=f"g{gi}")
                            nc.scalar.activation(at[:], ps[:], func,
                                                 bias=biasT[:, m:m + 1])
                            acts.append(at)
                        sig_i, sig_f, tanh_g, sig_o = acts
                        cs = cT[:, hb, c0:c0 + CH]
                        t1 = dvet.tile([128, CH], f32, tag="t1")
                        nc.vector.tensor_tensor(t1[:], sig_f[:], cs, OP.mult)
                        prod = dvet.tile([128, CH], bf16, tag="prod")
                        nc.vector.tensor_tensor(prod[:], sig_i[:], tanh_g[:], OP.mult)
                        nc.vector.tensor_tensor(cs, t1[:], prod[:], OP.add)
                        tnc = dvet.tile([128, CH], bf16, tag="tanhc")
                        nc.scalar.activation(tnc[:], cs, AF.Tanh)
                        nc.vector.tensor_tensor(sT[:, hb, c0:c0 + CH],
                                                sig_o[:], tnc[:], OP.mult)
                # y_t = conv_w . h_t + conv_b
                for j in range(N512):
                    cps = cpsp.tile([1, 512], f32, tag="cps")
                    for k in range(2):
                        nc.tensor.matmul(
                            cps[:],
                            cwT[:, k:k + 1],
                            sT[:, k, j * 512:(j + 1) * 512],
                            start=(k == 0),
                            stop=(k == 1),
                        )
                    ysb = ysbp.tile([1, 512], f32, tag="ysb")
                    nc.scalar.activation(ysb[:], cps[:], AF.Identity,
                                         bias=cbT[:, 0:1])
                    nc.sync.dma_start(y[t:t + 1, j * 512:(j + 1) * 512], ysb[:])

    nc.finalize()
    return nc


def _get_nc():
    if "nc" not in _CACHE:
        _CACHE["nc"] = _build_nc()
    return _CACHE["nc"]


def kernel(x, W_ih, W_hh, b_ih, b_hh, conv_w, conv_b):
    from concourse.bass_utils import run_bass_kernel_spmd

    x = np.ascontiguousarray(np.asarray(x, dtype=np.float32))
    # host-side shard prep: x_last extraction (pure indexing) + transpose
    x_last = x.transpose(0, 2, 1, 3).reshape(BN, F * S)[:, (S - 1) * F:]  # (BN,128)
    xlT = np.ascontiguousarray(x_last.T)                                  # (128, BN)

    Wc = np.concatenate([np.asarray(W_hh), np.asarray(W_ih)], axis=1)     # (1024,384)
    wT = np.ascontiguousarray(Wc.T, dtype=np.float32)                     # (384,1024)
    bias = np.asarray(b_ih, np.float32) + np.asarray(b_hh, np.float32)    # (1024,)
    cw = np.ascontiguousarray(np.asarray(conv_w, np.float32))             # (256,)
    cb = np.asarray(conv_b, np.float32).reshape(1, 1)                     # (1,1)

    in_maps = []
    for c in range(NCORES):
        in_maps.append({
            "xT": np.ascontiguousarray(xlT[:, c * R:(c + 1) * R]),
            "wT": wT,
            "biasd": bias,
            "convw": cw,
            "convb": cb,
        })

    nc = _get_nc()
    trace = os.environ.get("KERNEL_TRACE", "0") == "1"
    res = run_bass_kernel_spmd(nc, in_maps, list(range(NCORES)), trace=trace)
    if trace and res.exec_time_ns is not None:
        print(f"HW exec time: {res.exec_time_ns} ns")
        _CACHE["exec_time_ns"] = res.exec_time_ns

    ys = np.concatenate([res.results[c]["y"] for c in range(NCORES)], axis=1)
    predict = np.ascontiguousarray(ys.T).reshape(B, N, STEPS).astype(np.float32)
    return (predict, predict)


# revision 7
# speedup vs baseline: 1.2339x; 1.2339x over previous
"""Trainium2 Bass kernel for nn_DecoderLSTM_8641474200440.

Computation (see original nn.Module):
  - Only the last 128 elements of each row's flattened (F,S) block feed the
    decoder ("x_last", shape (B*N, 128) after the torch reshape chain).
  - gx = x_last @ W_ih.T + b_ih + b_hh is constant across the 24 decode steps.
  - 24-step LSTM recurrence over 32768 independent rows, hidden 256.
  - y_t = h_t @ conv_w + conv_b  -> output (B, N, 24), returned twice.

Device strategy (8 cores, data-parallel over the 32768 rows, 4096 rows/core):
  - "transposed" layout: feature dims on SBUF partitions, rows on the free dim.
  - gx (+ biases) precomputed ONCE into SBUF bf16; each step does a K=256
    matmul sweep (h @ W_hh.T) into PSUM, then DVE adds gx while draining
    PSUM -> SBUF (the per-MM LDWEIGHTS cost on this toolchain makes extra
    matmul k-chunks far more expensive than a DVE add).
  - gates: ScalarE sigmoid/tanh, cell update on VectorE (+ h-mul offloaded
    to GPSIMD), c kept fp32, h stored bf16.
  - y_t via M=1 matmuls against conv_w, +conv_b via ACT Identity bias,
    interleaved per column chunk for pipelining.
"""

import os
import sys
import numpy as np

if "/opt/trn_rl_repo" not in sys.path:
    sys.path.insert(0, "/opt/trn_rl_repo")

B, F, N, S = 32, 128, 1024, 12
H, STEPS = 256, 24
BN = B * N
NCORES = 8
R = BN // NCORES          # rows per core = 4096
CH = 1024                 # free-dim chunk for ACT/DVE ops
NCH = R // CH
N512 = R // 512

_CACHE = {}


def _build_nc():
    import concourse.bass as bass
    import concourse.mybir as mybir
    import concourse.tile as tile
    from concourse import bacc

    f32 = mybir.dt.float32
    bf16 = mybir.dt.bfloat16
    AF = mybir.ActivationFunctionType
    OP = mybir.AluOpType

    nc = bacc.Bacc(None, target_bir_lowering=False)

    xT = nc.dram_tensor("xT", [F, R], f32, kind="ExternalInput")
    wT = nc.dram_tensor("wT", [384, 4 * H], f32, kind="ExternalInput")
    biasd = nc.dram_tensor("biasd", [4 * H], f32, kind="ExternalInput")
    convw = nc.dram_tensor("convw", [H], f32, kind="ExternalInput")
    convb = nc.dram_tensor("convb", [1, 1], f32, kind="ExternalInput")
    y = nc.dram_tensor("y", [STEPS, R], f32, kind="ExternalOutput")

    with tile.TileContext(nc) as tc:
        with (
            tc.tile_pool(name="const", bufs=1) as const,
            tc.tile_pool(name="gps", bufs=3, space="PSUM") as gps,
            tc.tile_pool(name="cps", bufs=2, space="PSUM") as cpsp,
            tc.tile_pool(name="gact", bufs=2) as gact,
            tc.tile_pool(name="dvet", bufs=2) as dvet,
            tc.tile_pool(name="ysbp", bufs=3) as ysbp,
        ):
            # ---- load + cast constants ----
            # wT rows: [0:256) = W_hh.T, [256:384) = W_ih.T
            wc = const.tile([128, 3, 4 * H], bf16)
            sT = const.tile([128, 2, R], bf16)      # [h0 | h1]
            nc.gpsimd.memset(sT[:], 0.0)
            cT = const.tile([128, 2, R], f32)
            nc.gpsimd.memset(cT[:], 0.0)
            biasT = const.tile([128, 8], f32)
            nc.sync.dma_start(biasT[:], biasd.rearrange("(o p) -> p o", p=128))
            cwT = const.tile([128, 2], bf16)
            cbT = const.tile([1, 1], f32)
            nc.sync.dma_start(cbT[:], convb[:])
            gx = const.tile([128, 8, R], bf16)

            with tc.tile_pool(name="xpool", bufs=1) as xpool:
                xbf = xpool.tile([128, R], bf16)
                with tc.tile_pool(name="stage", bufs=1) as stage:
                    wstage = stage.tile([128, 3, 4 * H], f32)
                    nc.sync.dma_start(wstage[:],
                                      wT.rearrange("(kc p) m -> p kc m", p=128))
                    nc.vector.tensor_copy(wc[:], wstage[:])
                    cwstage = stage.tile([128, 2], f32)
                    nc.sync.dma_start(cwstage[:],
                                      convw.rearrange("(o p) -> p o", p=128))
                    nc.vector.tensor_copy(cwT[:], cwstage[:])
                    for ch in range(NCH):
                        xstage = stage.tile([128, CH], f32, tag="xs")
                        nc.sync.dma_start(xstage[:], xT[:, ch * CH:(ch + 1) * CH])
                        nc.vector.tensor_copy(xbf[:, ch * CH:(ch + 1) * CH],
                                              xstage[:])

                # ---- precompute gx = x @ W_ih.T + b (bf16, per m-tile) ----
                for m in range(8):
                    for ch in range(NCH):
                        c0 = ch * CH
                        ps = gps.tile([128, CH], f32, tag="gps")
                        for h2 in range(CH // 512):
                            nc.tensor.matmul(
                                ps[:, h2 * 512:(h2 + 1) * 512],
                                wc[:, 2, m * 128:(m + 1) * 128],
                                xbf[:, c0 + h2 * 512: c0 + (h2 + 1) * 512],
                                start=True, stop=True,
                            )
                        nc.vector.tensor_scalar(
                            gx[:, m, c0:c0 + CH], ps[:], biasT[:, m:m + 1], None,
                            OP.add)

            # ---- recurrence ----
            for t in range(STEPS):
                for ch in range(NCH):
                    c0 = ch * CH
                    for hb in range(2):
                        acts = []
                        for gi, m in enumerate((hb, 2 + hb, 4 + hb, 6 + hb)):
                            ps = gps.tile([128, CH], f32, tag="gps")
                            for k in range(2):
                                for h2 in range(CH // 512):
                                    nc.tensor.matmul(
                                        ps[:, h2 * 512:(h2 + 1) * 512],
                                        wc[:, k, m * 128:(m + 1) * 128],
                                        sT[:, k, c0 + h2 * 512: c0 + (h2 + 1) * 512],
                                        start=(k == 0),
                                        stop=(k == 1),
                                    )
                            nc.vector.tensor_tensor(ps[:], ps[:],
                                                    gx[:, m, c0:c0 + CH], OP.add)
                            func = AF.Tanh if gi == 2 else AF.Sigmoid
                            dt_g = f32 if gi == 1 else bf16
                            at = gact.tile([128, CH], dt_g, tag=f"g{gi}")
                            nc.scalar.activation(at[:], ps[:], func)
                            acts.append(at)
                        sig_i, sig_f, tanh_g, sig_o = acts
                        cs = cT[:, hb, c0:c0 + CH]
                        t1 = dvet.tile([128, CH], f32, tag="t1")
                        nc.vector.tensor_tensor(t1[:], sig_f[:], cs, OP.mult)
                        prod = dvet.tile([128, CH], bf16, tag="prod")
                        nc.vector.tensor_tensor(prod[:], sig_i[:], tanh_g[:], OP.mult)
                        nc.vector.tensor_tensor(cs, t1[:], prod[:], OP.add)
                        tnc = dvet.tile([128, CH], bf16, tag="tanhc")
                        nc.scalar.activation(tnc[:], cs, AF.Tanh)
                        nc.gpsimd.tensor_tensor(sT[:, hb, c0:c0 + CH],
                                                sig_o[:], tnc[:], OP.mult)
                    # conv for the two 512-col chunks of this ch
                    for j in range(ch * (CH // 512), (ch + 1) * (CH // 512)):
                        cps = cpsp.tile([1, 512], f32, tag="cps")
                        for k in range(2):
                            nc.tensor.matmul(
                                cps[:],
                                cwT[:, k:k + 1],
                                sT[:, k, j * 512:(j + 1) * 512],
                                start=(k == 0),
                                stop=(k == 1),
                            )
                        ysb = ysbp.tile([1, 512], f32, tag="ysb")
                        nc.scalar.activation(ysb[:], cps[:], AF.Identity,
                                             bias=cbT[:, 0:1])
                        nc.sync.dma_start(y[t:t + 1, j * 512:(j + 1) * 512], ysb[:])

    nc.finalize()
    return nc


def _get_nc():
    if "nc" not in _CACHE:
        _CACHE["nc"] = _build_nc()
    return _CACHE["nc"]


def kernel(x, W_ih, W_hh, b_ih, b_hh, conv_w, conv_b):
    from concourse.bass_utils import run_bass_kernel_spmd

    x = np.ascontiguousarray(np.asarray(x, dtype=np.float32))
    # host-side shard prep: x_last extraction (pure indexing) + transpose
    x_last = x.transpose(0, 2, 1, 3).reshape(BN, F * S)[:, (S - 1) * F:]  # (BN,128)
    xlT = np.ascontiguousarray(x_last.T)                                  # (128, BN)

    Wc = np.concatenate([np.asarray(W_hh), np.asarray(W_ih)], axis=1)     # (1024,384)
    wT = np.ascontiguousarray(Wc.T, dtype=np.float32)                     # (384,1024)
    bias = np.asarray(b_ih, np.float32) + np.asarray(b_hh, np.float32)    # (1024,)
    cw = np.ascontiguousarray(np.asarray(conv_w, np.float32))             # (256,)
    cb = np.asarray(conv_b, np.float32).reshape(1, 1)                     # (1,1)

    in_maps = []
    for c in range(NCORES):
        in_maps.append({
            "xT": np.ascontiguousarray(xlT[:, c * R:(c + 1) * R]),
            "wT": wT,
            "biasd": bias,
            "convw": cw,
            "convb": cb,
        })

    nc = _get_nc()
    res = run_bass_kernel_spmd(nc, in_maps, list(range(NCORES)))

    ys = np.concatenate([res.results[c]["y"] for c in range(NCORES)], axis=1)
    predict = np.ascontiguousarray(ys.T).reshape(B, N, STEPS).astype(np.float32)
    return (predict, predict)


# revision 10
# speedup vs baseline: 2.1956x; 1.7795x over previous
"""Trainium2 Bass kernel for nn_DecoderLSTM_8641474200440.

Computation (see original nn.Module):
  - Only the last 128 elements of each row's flattened (F,S) block feed the
    decoder ("x_last", shape (B*N, 128) after the torch reshape chain).
  - gx = x_last @ W_ih.T + b_ih + b_hh is constant across the 24 decode steps.
  - 24-step LSTM recurrence over 32768 independent rows, hidden 256.
  - y_t = h_t @ conv_w + conv_b  -> output (B, N, 24), returned twice.

Device strategy (8 cores, data-parallel over the 32768 rows, 4096 rows/core):
  - "transposed" layout: feature dims on SBUF partitions, rows on the free dim.
  - gx (+ biases) precomputed ONCE into SBUF bf16; each step does a K=256
    matmul sweep (h @ W_hh.T) into PSUM, then DVE adds gx while draining
    PSUM -> SBUF (the per-MM LDWEIGHTS cost on this toolchain makes extra
    matmul k-chunks far more expensive than a DVE add).
  - gates: ScalarE sigmoid/tanh, cell update on VectorE (+ h-mul offloaded
    to GPSIMD), c kept fp32, h stored bf16.
  - y_t via M=1 matmuls against conv_w, +conv_b via ACT Identity bias,
    interleaved per column chunk for pipelining.
"""

import os
import sys
import numpy as np

if "/opt/trn_rl_repo" not in sys.path:
    sys.path.insert(0, "/opt/trn_rl_repo")

B, F, N, S = 32, 128, 1024, 12
H, STEPS = 256, 24
BN = B * N
NCORES = 8
R = BN // NCORES          # rows per core = 4096
CH = 1024                 # free-dim chunk for ACT/DVE ops
NCH = R // CH
N512 = R // 512

_CACHE = {}


def _build_nc():
    import concourse.bass as bass
    import concourse.mybir as mybir
    import concourse.tile as tile
    from concourse import bacc

    f32 = mybir.dt.float32
    bf16 = mybir.dt.bfloat16
    AF = mybir.ActivationFunctionType
    OP = mybir.AluOpType

    nc = bacc.Bacc(None, target_bir_lowering=False)

    xT = nc.dram_tensor("xT", [F, R], f32, kind="ExternalInput")
    wT = nc.dram_tensor("wT", [384, 4 * H], f32, kind="ExternalInput")
    biasd = nc.dram_tensor("biasd", [4 * H], f32, kind="ExternalInput")
    convw = nc.dram_tensor("convw", [H], f32, kind="ExternalInput")
    convb = nc.dram_tensor("convb", [1, 1], f32, kind="ExternalInput")
    y = nc.dram_tensor("y", [STEPS, R], f32, kind="ExternalOutput")

    with tile.TileContext(nc) as tc:
        with (
            tc.tile_pool(name="const", bufs=1) as const,
            tc.tile_pool(name="gps", bufs=3, space="PSUM") as gps,
            tc.tile_pool(name="cps", bufs=2, space="PSUM") as cpsp,
            tc.tile_pool(name="gact", bufs=2) as gact,
            tc.tile_pool(name="dvet", bufs=2) as dvet,
            tc.tile_pool(name="ysbp", bufs=3) as ysbp,
        ):
            # ---- load + cast constants ----
            # wT rows: [0:256) = W_hh.T, [256:384) = W_ih.T
            wc = const.tile([128, 3, 4 * H], bf16)
            sT = const.tile([128, 2, R], bf16)      # [h0 | h1]
            nc.gpsimd.memset(sT[:], 0.0)
            cT = const.tile([128, 2, R], f32)
            nc.gpsimd.memset(cT[:], 0.0)
            biasT = const.tile([128, 8], f32)
            nc.sync.dma_start(biasT[:], biasd.rearrange("(o p) -> p o", p=128))
            cwT = const.tile([128, 2], bf16)
            cbT = const.tile([1, 1], f32)
            nc.sync.dma_start(cbT[:], convb[:])
            gx = const.tile([128, 8, R], bf16)

            with tc.tile_pool(name="xpool", bufs=1) as xpool:
                xbf = xpool.tile([128, R], bf16)
                with tc.tile_pool(name="stage", bufs=1) as stage:
                    wstage = stage.tile([128, 3, 4 * H], f32)
                    nc.sync.dma_start(wstage[:],
                                      wT.rearrange("(kc p) m -> p kc m", p=128))
                    nc.vector.tensor_copy(wc[:], wstage[:])
                    cwstage = stage.tile([128, 2], f32)
                    nc.sync.dma_start(cwstage[:],
                                      convw.rearrange("(o p) -> p o", p=128))
                    nc.vector.tensor_copy(cwT[:], cwstage[:])
                    for ch in range(NCH):
                        xstage = stage.tile([128, CH], f32, tag="xs")
                        nc.sync.dma_start(xstage[:], xT[:, ch * CH:(ch + 1) * CH])
                        nc.vector.tensor_copy(xbf[:, ch * CH:(ch + 1) * CH],
                                              xstage[:])

                # ---- precompute gx = x @ W_ih.T + b (bf16, per m-tile) ----
                for m in range(8):
                    for ch in range(NCH):
                        c0 = ch * CH
                        ps = gps.tile([128, CH], f32, tag="gps")
                        for h2 in range(CH // 512):
                            nc.tensor.matmul(
                                ps[:, h2 * 512:(h2 + 1) * 512],
                                wc[:, 2, m * 128:(m + 1) * 128],
                                xbf[:, c0 + h2 * 512: c0 + (h2 + 1) * 512],
                                start=True, stop=True,
                            )
                        nc.vector.tensor_scalar(
                            gx[:, m, c0:c0 + CH], ps[:], biasT[:, m:m + 1], None,
                            OP.add)

            # ---- recurrence ----
            for t in range(STEPS):
                for ch in range(NCH):
                    c0 = ch * CH
                    for hb in range(2):
                        acts = []
                        for gi, m in enumerate((hb, 2 + hb, 4 + hb, 6 + hb)):
                            ps = gps.tile([128, CH], f32, tag="gps")
                            for k in range(2):
                                for h2 in range(CH // 512):
                                    nc.tensor.matmul(
                                        ps[:, h2 * 512:(h2 + 1) * 512],
                                        wc[:, k, m * 128:(m + 1) * 128],
                                        sT[:, k, c0 + h2 * 512: c0 + (h2 + 1) * 512],
                                        start=(k == 0),
                                        stop=(k == 1),
                                    )
                            nc.vector.tensor_tensor(ps[:], ps[:],
                                                    gx[:, m, c0:c0 + CH], OP.add)
                            func = AF.Tanh if gi == 2 else AF.Sigmoid
                            dt_g = f32 if gi == 1 else bf16
                            at = gact.tile([128, CH], dt_g, tag=f"g{gi}")
                            nc.scalar.activation(at[:], ps[:], func)
                            acts.append(at)
                        sig_i, sig_f, tanh_g, sig_o = acts
                        cs = cT[:, hb, c0:c0 + CH]
                        t1 = dvet.tile([128, CH], f32, tag="t1")
                        nc.vector.tensor_tensor(t1[:], sig_f[:], cs, OP.mult)
                        prod = dvet.tile([128, CH], bf16, tag="prod")
                        nc.vector.tensor_tensor(prod[:], sig_i[:], tanh_g[:], OP.mult)
                        nc.vector.tensor_tensor(cs, t1[:], prod[:], OP.add)
                        tnc = dvet.tile([128, CH], bf16, tag="tanhc")
                        nc.scalar.activation(tnc[:], cs, AF.Tanh)
                        nc.gpsimd.tensor_tensor(sT[:, hb, c0:c0 + CH],
                                                sig_o[:], tnc[:], OP.mult)
                    # conv for the two 512-col chunks of this ch
                    for j in range(ch * (CH // 512), (ch + 1) * (CH // 512)):
                        cps = cpsp.tile([1, 512], f32, tag="cps")
                        for k in range(2):
                            nc.tensor.matmul(
                                cps[:],
                                cwT[:, k:k + 1],
                                sT[:, k, j * 512:(j + 1) * 512],
                                start=(k == 0),
                                stop=(k == 1),
                            )
                        ysb = ysbp.tile([1, 512], f32, tag="ysb")
                        nc.scalar.activation(ysb[:], cps[:], AF.Identity,
                                             bias=cbT[:, 0:1])
                        nc.sync.dma_start(y[t:t + 1, j * 512:(j + 1) * 512], ysb[:])

    nc.finalize()
    return nc


def _get_nc():
    if "nc" not in _CACHE:
        _CACHE["nc"] = _build_nc()
    return _CACHE["nc"]


def kernel(x, W_ih, W_hh, b_ih, b_hh, conv_w, conv_b):
    from concourse.bass_utils import run_bass_kernel_spmd

    x = np.ascontiguousarray(np.asarray(x, dtype=np.float32))
    # host-side shard prep: x_last extraction (pure indexing) + transpose
    x_last = x.transpose(0, 2, 1, 3).reshape(BN, F * S)[:, (S - 1) * F:]  # (BN,128)
    xlT = np.ascontiguousarray(x_last.T)                                  # (128, BN)

    Wc = np.concatenate([np.asarray(W_hh), np.asarray(W_ih)], axis=1)     # (1024,384)
    wT = np.ascontiguousarray(Wc.T, dtype=np.float32)                     # (384,1024)
    bias = np.asarray(b_ih, np.float32) + np.asarray(b_hh, np.float32)    # (1024,)
    cw = np.ascontiguousarray(np.asarray(conv_w, np.float32))             # (256,)
    cb = np.asarray(conv_b, np.float32).reshape(1, 1)                     # (1,1)

    in_maps = []
    for c in range(NCORES):
        in_maps.append({
            "xT": np.ascontiguousarray(xlT[:, c * R:(c + 1) * R]),
            "wT": wT,
            "biasd": bias,
            "convw": cw,
            "convb": cb,
        })

    nc = _get_nc()
    res = run_bass_kernel_spmd(nc, in_maps, list(range(NCORES)))

    ys = np.concatenate([res.results[c]["y"] for c in range(NCORES)], axis=1)
    predict = np.ascontiguousarray(ys.T).reshape(B, N, STEPS).astype(np.float32)
    return (predict, predict)
